# revision 1
# baseline (speedup 1.0000x reference)
"""Trainium2 Bass kernel for nn_FDSM_40295383171690.

Math (verified vs reference in fp64, rel err ~4e-7):
  gating: GN(concat(x,x)) == concat(GN4(x), GN4(x)); fold gamma/beta into the
          1x1 conv -> W', b'; weights = softmax(wg @ GAP(relu(W' xn + b')))
  fft:    out = irfft2( rfft2(x)^2 * Wmix ) + r*x
          Wmix[b] = sum_f weights[b,f] * Wsym[f],  Wsym = ds_w filters with
          columns k2 in {0,64} Hermitian-symmetrized along k1 (this absorbs
          the rfft2(irfft2(.)) Hermitian projection exactly).

Sharding: core k = gating for sample k (all C) + FFT branch for channels
[8k,8k+8) of all samples; the [8,4] gating weights are AllGathered on-chip.

DFTs are dense matmuls: stage1 (contract h, lhsT=x, rhs=[Ch|Sh], fp32r N=256),
stage2 (contract w, lhsT=U/V bf16, rhs=[Cw|-Sw],[-Sw|-Cw]), iDFT-A (contract
k1, lhsT=D fp32r, rhs=[Cih|Sih],[-Sih|Cih] N=256), iDFT-B (contract k2,
lhsT=Z2 bf16, rhs=Gc,Gs). Wmix is a K=(f x 32)-packed matmul with col-tiling.
"""

import numpy as np
import ml_dtypes

import concourse.bass as bass
import concourse.bacc as bacc
import concourse.mybir as mybir
import concourse.tile as tile
from concourse.bass_utils import run_bass_kernel_spmd

dt = mybir.dt
AF = mybir.ActivationFunctionType
ALU = mybir.AluOpType
AX = mybir.AxisListType

B, C, H, W, F = 8, 64, 128, 128, 4
WF = 65
NCORES = 8
CS = C // NCORES
EPS = 1e-5
HW = H * W

_cache = {}
DEBUG = False
N_B = 8
SIM_MODE = False


def _build_constants():
    h = np.arange(H)
    k1 = np.arange(H)
    w = np.arange(W)
    k2 = np.arange(WF)
    Ch = np.cos(2 * np.pi * np.outer(h, k1) / H).astype(np.float32)
    Sh = np.sin(2 * np.pi * np.outer(h, k1) / H).astype(np.float32)
    Cw = np.cos(2 * np.pi * np.outer(w, k2) / W).astype(np.float32)
    Sw = np.sin(2 * np.pi * np.outer(w, k2) / W).astype(np.float32)
    Cih = (np.cos(2 * np.pi * np.outer(k1, h) / H) / H).astype(np.float32)
    Sih = (np.sin(2 * np.pi * np.outer(k1, h) / H) / H).astype(np.float32)
    cj = np.ones(WF, np.float32)
    cj[1:64] = 2.0
    Gc = (cj[:, None] * np.cos(2 * np.pi * np.outer(k2, w) / W) / W).astype(np.float32)
    Gs = (-cj[:, None] * np.sin(2 * np.pi * np.outer(k2, w) / W) / W).astype(np.float32)

    bf = ml_dtypes.bfloat16
    consts = {
        "R1": np.concatenate([Ch, Sh], 1),
        "R2a": np.concatenate([Cw, -Sw], 1),
        "R2b": np.concatenate([-Sw, -Cw], 1),
        "RA1": np.concatenate([Cih, Sih], 1),
        "RA2": np.concatenate([-Sih, Cih], 1),
        "RB1": Gc,
        "RB2": Gs,
    }
    G16 = np.zeros((128, 16), np.float32)
    E16 = np.zeros((16, 128), np.float32)
    for p in range(128):
        g = (p % 64) // 4
        G16[p, g] = 1.0
        E16[g, p] = 1.0
    F2 = np.zeros((128, 64), np.float32)
    for p in range(128):
        F2[p, p % 64] = 1.0 / HW
    E4 = np.zeros((4, 128), np.float32)
    for p in range(128):
        E4[p // 32, p] = 1.0
    maskJ = np.zeros((4, 128, 128), np.float32)
    for J in range(4):
        for p in range(128):
            f, pp = p // 32, p % 32
            maskJ[J, p, 32 * J + pp] = 1.0
    consts.update({"G16": G16, "E16": E16, "F2": F2, "E4": E4,
                   "maskJ": maskJ})
    return consts


def _prep_params(inputs):
    gamma = np.asarray(inputs["gn_gamma"], np.float64)
    beta = np.asarray(inputs["gn_beta"], np.float64)
    agg_w = np.asarray(inputs["agg_w"], np.float64)
    agg_b = np.asarray(inputs["agg_b"], np.float64)
    wg_w = np.asarray(inputs["wg_w"], np.float64)
    wg_b = np.asarray(inputs["wg_b"], np.float64)

    Wp = agg_w[:, :C] * gamma[None, :C] + agg_w[:, C:] * gamma[None, C:]
    bp = agg_w[:, :C] @ beta[:C] + agg_w[:, C:] @ beta[C:] + agg_b
    Wblk = np.zeros((128, 128), np.float32)
    for t in range(2):
        Wblk[64 * t:64 * t + 64, 64 * t:64 * t + 64] = Wp.T.astype(np.float32)
    bprime = np.zeros((128, 1), np.float32)
    bprime[:64, 0] = bp.astype(np.float32)
    bprime[64:, 0] = bp.astype(np.float32)
    WgT = wg_w.T.astype(np.float32)
    wgb = wg_b.astype(np.float32).reshape(1, 4)

    ds = np.asarray(inputs["ds_w"], np.float64)
    Wc = ds[..., 0] + 1j * ds[..., 1]                     # [F,C,H(k1),WF(k2)]
    rev = (-np.arange(H)) % H
    Wt = Wc.copy()
    for j in (0, WF - 1):
        Wt[..., j] = 0.5 * (Wc[..., j] + np.conj(Wc[:, :, rev, j]))
    rw = float(np.asarray(inputs["residual_weight"]).ravel()[0])
    return Wblk, bprime, WgT, wgb, Wt, rw


def _build_kernel():
    bf16, f32, f32r = dt.bfloat16, dt.float32, dt.float32r

    nc = bacc.Bacc("TRN2", target_bir_lowering=False, debug=False,
                   num_devices=NCORES)

    d = {}
    d["featf"] = nc.dram_tensor("featf", [128, B * CS * W], f32r,
                                kind="ExternalInput").ap()
    d["featg"] = nc.dram_tensor("featg", [128, 64 * 128], f32,
                                kind="ExternalInput").ap()
    d["ftiles"] = nc.dram_tensor("ftiles", [4, 128, CS * 2 * WF], f32r,
                                 kind="ExternalInput").ap()
    d["maskJ"] = nc.dram_tensor("maskJ", [4, 128, 128], f32,
                                kind="ExternalInput").ap()
    for name, shape, dty in [
        ("R1", [128, 256], f32r), ("R2a", [128, 130], f32),
        ("R2b", [128, 130], f32), ("RA1", [128, 256], f32r),
        ("RA2", [128, 256], f32r), ("RB1", [65, 128], f32),
        ("RB2", [65, 128], f32), ("G16", [128, 16], f32),
        ("E16", [16, 128], f32), ("F2", [128, 64], f32),
        ("E4", [4, 128], f32),
        ("Wblk", [128, 128], f32), ("bprime", [128, 1], f32),
        ("WgT", [64, 4], f32), ("wgb", [1, 4], f32),
        ("rcol", [128, 1], f32),
    ]:
        d[name] = nc.dram_tensor(name, shape, dty, kind="ExternalInput").ap()
    out_d = nc.dram_tensor("out", [B, CS, H, W], f32, kind="ExternalOutput").ap()
    if DEBUG:
        dbg = {
            "d_stats": nc.dram_tensor("d_stats", [128, 2], f32, kind="ExternalOutput").ap(),
            "d_gs": nc.dram_tensor("d_gs", [16, 6], f32, kind="ExternalOutput").ap(),
            "d_nstat": nc.dram_tensor("d_nstat", [128, 2], f32, kind="ExternalOutput").ap(),
            "d_gap": nc.dram_tensor("d_gap", [128, 16], f32, kind="ExternalOutput").ap(),
            "d_pooled": nc.dram_tensor("d_pooled", [64, 1], f32, kind="ExternalOutput").ap(),
            "d_logit": nc.dram_tensor("d_logit", [1, 8], f32, kind="ExternalOutput").ap(),
            "d_wrow": nc.dram_tensor("d_wrow", [1, 4], f32, kind="ExternalOutput").ap(),
            "d_wcol": nc.dram_tensor("d_wcol", [128, 8], f32, kind="ExternalOutput").ap(),
            "d_xn": nc.dram_tensor("d_xn", [128, 512], f32, kind="ExternalOutput").ap(),
            "d_y": nc.dram_tensor("d_y", [128, 512], f32, kind="ExternalOutput").ap(),
            "d_wb": nc.dram_tensor("d_wb", [128, 128], f32, kind="ExternalOutput").ap(),
            "d_wb2": nc.dram_tensor("d_wb2", [128, 128], f32, kind="ExternalOutput").ap(),
        }

    with tile.TileContext(nc) as tc:
        with (
            tc.tile_pool(name="consts", bufs=1) as cp,
            tc.tile_pool(name="feat", bufs=1) as fp,
            tc.tile_pool(name="gate", bufs=1) as gp,
            tc.tile_pool(name="work", bufs=3) as wkp,
            tc.tile_pool(name="sgrp", bufs=2) as sgp,
            tc.tile_pool(name="outp", bufs=3) as op_,
            tc.tile_pool(name="ps_a", bufs=2, space="PSUM") as ps_a,
            tc.tile_pool(name="ps_b", bufs=2, space="PSUM") as ps_b,
            tc.tile_pool(name="ps_c", bufs=2, space="PSUM") as ps_c,
            tc.tile_pool(name="ps_d", bufs=1, space="PSUM") as ps_d,
            tc.tile_pool(name="ps_m", bufs=1, space="PSUM") as ps_m,
            tc.tile_pool(name="dram", bufs=1, space="DRAM") as dr,
        ):
            ct = {}
            for name in ["R1", "R2a", "R2b", "RA1", "RA2", "RB1", "RB2",
                         "G16", "E16", "F2", "E4", "Wblk",
                         "bprime", "WgT", "wgb", "rcol"]:
                t = cp.tile(list(d[name].shape), d[name].dtype, tag=name)
                nc.sync.dma_start(t[:], d[name][:])
                ct[name] = t
            for name in ["R2a", "R2b", "RB1", "RB2"]:
                t = cp.tile(list(d[name].shape), bf16, tag=name + "b")
                nc.vector.tensor_copy(t[:], ct[name][:])
                ct[name] = t

            maskt = []
            for J in range(4):
                t = cp.tile([128, 128], f32, tag=f"maskJ{J}")
                nc.sync.dma_start(t[:], d["maskJ"][J])
                maskt.append(t)

            featb = []
            for b in range(B):
                t = fp.tile([128, CS * W], f32r, tag=f"featb{b}")
                nc.sync.dma_start(t[:], d["featf"][:, b * CS * W:(b + 1) * CS * W])
                featb.append(t)
            featg = fp.tile([128, 64 * 128], f32, tag="featg")
            nc.sync.dma_start(featg[:], d["featg"][:])
            ftl = []
            for J in range(4):
                t = fp.tile([128, CS * 2 * WF], f32r, tag=f"ftl{J}")
                nc.sync.dma_start(t[:], d["ftiles"][J])
                ftl.append(t)

            # ================= gating (sample b = core id) ===================
            xn = gp.tile([128, 64 * 128], f32r, tag="xn")
            wblkr = gp.tile([128, 128], f32r, tag="wblkr")
            nc.vector.tensor_copy(wblkr[:], ct["Wblk"][:])
            stats = gp.tile([128, 2], f32, tag="stats")
            nc.vector.tensor_scalar(xn[:], featg[:], 1.0, 0.0, ALU.mult,
                                    ALU.add, accum_out=stats[:, 0:1])
            nc.scalar.activation(xn[:], featg[:], AF.Square,
                                 accum_out=stats[:, 1:2])
            gstat = ps_m.tile([16, 2], f32, tag="pmix")
            nc.tensor.matmul(gstat[:], ct["G16"][:], stats[:])
            gs = gp.tile([16, 6], f32, tag="gs")
            nc.scalar.mul(gs[:, 0:1], gstat[:, 0:1], 1.0 / (4 * HW))
            nc.scalar.mul(gs[:, 1:2], gstat[:, 1:2], 1.0 / (4 * HW))
            nc.scalar.activation(gs[:, 2:3], gs[:, 0:1], AF.Square)
            nc.vector.tensor_sub(gs[:, 3:4], gs[:, 1:2], gs[:, 2:3])
            epst = gp.tile([16, 1], f32, tag="epst")
            nc.vector.memset(epst[:], EPS)
            nc.scalar.activation(gs[:, 4:5], gs[:, 3:4], AF.Sqrt,
                                 bias=epst[:, 0:1])
            nc.vector.reciprocal(gs[:, 5:6], gs[:, 4:5])
            gs2 = gp.tile([16, 2], f32, tag="gs2")
            nc.vector.tensor_mul(gs2[:, 0:1], gs[:, 0:1], gs[:, 5:6])
            nc.vector.tensor_scalar_mul(gs2[:, 0:1], gs2[:, 0:1], -1.0)
            nc.vector.tensor_copy(gs2[:, 1:2], gs[:, 5:6])
            pstat = ps_m.tile([128, 2], f32, tag="pmix")
            nc.tensor.matmul(pstat[:], ct["E16"][:], gs2[:])
            nstat = gp.tile([128, 2], f32, tag="nstat")
            nc.scalar.copy(nstat[:], pstat[:])
            nc.scalar.activation(xn[:], featg[:], AF.Identity,
                                 bias=nstat[:, 0:1], scale=nstat[:, 1:2])
            if DEBUG:
                wbf = gp.tile([128, 128], f32, tag="wbf")
                nc.vector.tensor_copy(wbf[:], wblkr[:])
                nc.sync.dma_start(dbg["d_wb"][:], wbf[:])
                xnf = gp.tile([128, 512], f32, tag="xnf")
                nc.vector.tensor_copy(xnf[:], xn[:, 0:512])
                nc.sync.dma_start(dbg["d_xn"][:], xnf[:])
            gap = gp.tile([128, 16], f32, tag="gap")
            for j in range(16):
                yp = ps_a.tile([128, 512], f32, tag="p1")
                nc.tensor.matmul(yp[:], wblkr[:],
                                 xn[:, j * 512:(j + 1) * 512])
                nc.scalar.activation(
                    featg[:, j * 512:(j + 1) * 512], yp[:], AF.Relu,
                    bias=ct["bprime"][:, 0:1], scale=1.0,
                    accum_out=gap[:, j:j + 1])
                if DEBUG and j == 0:
                    yf = gp.tile([128, 512], f32, tag="yf")
                    nc.scalar.copy(yf[:], yp[:])
                    nc.sync.dma_start(dbg["d_y"][:], yf[:])
            gsum = gp.tile([128, 1], f32, tag="gsum")
            nc.vector.tensor_reduce(gsum[:], gap[:], AX.X, ALU.add)
            ppool = ps_m.tile([64, 1], f32, tag="pmix")
            nc.tensor.matmul(ppool[:], ct["F2"][:], gsum[:])
            pooled = gp.tile([64, 1], f32, tag="pooled")
            nc.scalar.copy(pooled[:], ppool[:])
            plog = ps_m.tile([1, 4], f32, tag="pmix")
            nc.tensor.matmul(plog[:], pooled[:], ct["WgT"][:])
            logit = gp.tile([1, 8], f32, tag="logit")
            nc.vector.memset(logit[:], 0.0)
            nc.vector.tensor_add(logit[:, 0:4], plog[:], ct["wgb"][:])
            nc.vector.tensor_reduce(logit[:, 4:5], logit[:, 0:4], AX.X, ALU.max)
            nc.vector.tensor_scalar(logit[:, 0:4], logit[:, 0:4],
                                    logit[:, 4:5], None, ALU.subtract)
            wrow = gp.tile([1, 4], f32, tag="wrow")
            nc.scalar.activation(wrow[:], logit[:, 0:4], AF.Exp,
                                 accum_out=logit[:, 5:6])
            nc.vector.reciprocal(logit[:, 6:7], logit[:, 5:6])
            nc.vector.tensor_scalar(wrow[:], wrow[:], logit[:, 6:7], None,
                                    ALU.mult)
            ag_in = dr.tile([1, 4], f32)
            ag_out = dr.tile([8, 4], f32)
            nc.sync.dma_start(ag_in[:], wrow[:])
            if SIM_MODE:
                for _b in range(8):
                    nc.sync.dma_start(ag_out[_b:_b + 1, :], ag_in[:])
            else:
                nc.gpsimd.collective_compute(
                    "AllGather", ALU.bypass, ins=[ag_in.opt()],
                    outs=[ag_out.opt()],
                    replica_groups=[list(range(NCORES))],
                )
            wT = gp.tile([4, 8], f32, tag="wT")
            nc.sync.dma_start(wT[:], ag_out[:].rearrange("b f -> f b"))
            pwcol = ps_m.tile([128, 8], f32, tag="pmix")
            nc.tensor.matmul(pwcol[:], ct["E4"][:], wT[:])
            wcol = gp.tile([128, 8], f32, tag="wcol")
            nc.scalar.copy(wcol[:], pwcol[:])
            if DEBUG:
                nc.sync.dma_start(dbg["d_stats"][:], stats[:])
                nc.sync.dma_start(dbg["d_gs"][:], gs[:])
                nc.sync.dma_start(dbg["d_nstat"][:], nstat[:])
                nc.sync.dma_start(dbg["d_gap"][:], gap[:])
                nc.sync.dma_start(dbg["d_pooled"][:], pooled[:])
                nc.sync.dma_start(dbg["d_logit"][:], logit[:])
                nc.sync.dma_start(dbg["d_wrow"][:], wrow[:])
                nc.sync.dma_start(dbg["d_wcol"][:], wcol[:])
            wpat = []
            for b in range(B):
                row = []  # d_wb2 dump appended after wpat build below
                for J in range(4):
                    t = gp.tile([128, 128], f32r, tag=f"wpat{b}_{J}")
                    nc.scalar.activation(t[:], maskt[J][:], AF.Identity,
                                         scale=wcol[:, b:b + 1])
                    row.append(t)
                wpat.append(row)

            if DEBUG:
                zz = gp.tile([128, 128], bf16, tag="zz")
                nc.vector.tensor_scalar(zz[:], wpat[7][3][:], 0.0, None, ALU.mult)
                wb2 = gp.tile([128, 128], f32, tag="wb2")
                nc.vector.tensor_add(wb2[:], ct["Wblk"][:], zz[:])
                nc.sync.dma_start(dbg["d_wb2"][:], wb2[:])
            # ================= FFT branch ====================================
            for b in range(N_B):
                fb = featb[b]
                for g in range(2):                      # 4-chain groups
                    c0 = 4 * g
                    Sr4 = sgp.tile([128, 260], f32, tag="Sr4")
                    Si4 = sgp.tile([128, 260], f32, tag="Si4")
                    Xi4 = sgp.tile([128, 260], f32, tag="Xi4")
                    Dr4 = sgp.tile([128, 260], f32r, tag="Dr4")
                    Di4 = sgp.tile([128, 260], f32r, tag="Di4")
                    Wm4 = sgp.tile([128, 520], f32, tag="Wm4")
                    m1 = sgp.tile([128, 260], f32, tag="m1")
                    m2 = sgp.tile([128, 260], f32, tag="m2")
                    m3 = sgp.tile([128, 260], f32, tag="m3")
                    m4 = sgp.tile([128, 260], f32, tag="m4")
                    pB = ps_d.tile([128, 512], f32, tag="pB")
                    for cc in range(2):                 # 2-chain psum subgroups
                        ch2 = c0 + 2 * cc
                        pm = ps_m.tile([128, 260], f32, tag="pmix")
                        for J in range(4):
                            nc.tensor.matmul(
                                pm[:], wpat[b][J][:],
                                ftl[J][:, ch2 * 130:(ch2 + 2) * 130],
                                start=(J == 0), stop=(J == 3))
                        p1 = ps_a.tile([128, 512], f32, tag="p1")
                        for j in range(2):
                            c = ch2 + j
                            nc.tensor.matmul(p1[:, j * 256:(j + 1) * 256],
                                             fb[:, c * 128:(c + 1) * 128],
                                             ct["R1"][:])
                        uv = wkp.tile([128, 512], bf16, tag="uv")
                        if cc == 0:
                            nc.vector.tensor_copy(uv[:], p1[:])
                        else:
                            nc.scalar.copy(uv[:], p1[:])
                        p2 = ps_b.tile([128, 260], f32, tag="p2")
                        for j in range(2):
                            nc.tensor.matmul(p2[:, j * 130:(j + 1) * 130],
                                             uv[:, j * 256:j * 256 + 128],
                                             ct["R2a"][:], start=True,
                                             stop=False)
                            nc.tensor.matmul(p2[:, j * 130:(j + 1) * 130],
                                             uv[:, j * 256 + 128:(j + 1) * 256],
                                             ct["R2b"][:], start=False,
                                             stop=True)
                        # strided views: [128, 2chain, 65]
                        p2v = p2[:].rearrange("p (j x) -> p j x", j=2)
                        xr = p2v[:, :, 0:65]
                        xi = p2v[:, :, 65:130]
                        s4 = slice(2 * cc, 2 * cc + 2)
                        srv = Sr4[:].rearrange("p (q x) -> p q x", q=4)[:, s4]
                        siv = Si4[:].rearrange("p (q x) -> p q x", q=4)[:, s4]
                        xiv = Xi4[:].rearrange("p (q x) -> p q x", q=4)[:, s4]
                        m1v = m1[:].rearrange("p (q x) -> p q x", q=4)[:, s4]
                        nc.scalar.activation(srv, xr, AF.Square)   # Xr^2
                        nc.vector.tensor_copy(xiv, xi)             # Xi
                        nc.scalar.activation(m1v, xi, AF.Square)   # Xi^2 (scratch)
                        # Si = 2*Xr*Xi  (one psum operand)
                        nc.vector.scalar_tensor_tensor(siv, xr, 2.0, xiv,
                                                       ALU.mult, ALU.mult)
                        # Sr = Xr^2 - Xi^2
                        nc.vector.tensor_sub(srv, srv, m1v)
                        nc.scalar.copy(Wm4[:, cc * 260:(cc + 1) * 260], pm[:])
                    # ---- D = S * Wmix  (4 chains batched) ----
                    wmv = Wm4[:].rearrange("p (q x) -> p q x", q=4)
                    wmr = wmv[:, :, 0:65]
                    wmi = wmv[:, :, 65:130]
                    sr_f = Sr4[:].rearrange("p (q x) -> p q x", q=4)
                    si_f = Si4[:].rearrange("p (q x) -> p q x", q=4)
                    m1f = m1[:].rearrange("p (q x) -> p q x", q=4)
                    m2f = m2[:].rearrange("p (q x) -> p q x", q=4)
                    m3f = m3[:].rearrange("p (q x) -> p q x", q=4)
                    m4f = m4[:].rearrange("p (q x) -> p q x", q=4)
                    nc.vector.tensor_mul(m1f, sr_f, wmr)
                    nc.vector.tensor_mul(m2f, si_f, wmi)
                    nc.gpsimd.tensor_mul(m3f, sr_f, wmi)
                    nc.gpsimd.tensor_mul(m4f, si_f, wmr)
                    nc.vector.tensor_sub(Dr4[:], m1[:], m2[:])
                    nc.gpsimd.tensor_add(Di4[:], m3[:], m4[:])
                    # ---- iDFT ----
                    for cc in range(2):
                        pA = ps_c.tile([65, 512], f32, tag="pA")
                        for j in range(2):
                            q = 2 * cc + j
                            nc.tensor.matmul(pA[:, j * 256:(j + 1) * 256],
                                             Dr4[:, q * 65:(q + 1) * 65],
                                             ct["RA1"][:], start=True,
                                             stop=False)
                            nc.tensor.matmul(pA[:, j * 256:(j + 1) * 256],
                                             Di4[:, q * 65:(q + 1) * 65],
                                             ct["RA2"][:], start=False,
                                             stop=True)
                        z2 = wkp.tile([65, 512], bf16, tag="z2")
                        if cc == 0:
                            nc.vector.tensor_copy(z2[:], pA[:])
                        else:
                            nc.scalar.copy(z2[:], pA[:])
                        for j in range(2):
                            q = 2 * cc + j
                            nc.tensor.matmul(pB[:, q * 128:(q + 1) * 128],
                                             z2[:, j * 256:j * 256 + 128],
                                             ct["RB1"][:], start=True,
                                             stop=False)
                            nc.tensor.matmul(pB[:, q * 128:(q + 1) * 128],
                                             z2[:, j * 256 + 128:(j + 1) * 256],
                                             ct["RB2"][:], start=False,
                                             stop=True)
                    ot = op_.tile([128, 512], f32, tag="ot")
                    nc.vector.scalar_tensor_tensor(
                        ot[:], fb[:, c0 * 128:(c0 + 4) * 128].bitcast(f32),
                        ct["rcol"][:, 0:1], pB[:], ALU.mult, ALU.add)
                    nc.sync.dma_start(
                        out_d[b, c0:c0 + 4].rearrange("c h w -> h c w"),
                        ot[:].rearrange("p (c w) -> p c w", c=4))
    nc.compile()
    return nc


def _get_kernel():
    if "nc" not in _cache:
        _cache["nc"] = _build_kernel()
        _cache["consts"] = _build_constants()
    return _cache["nc"], _cache["consts"]


def kernel(**inputs):
    nc, consts = _get_kernel()
    Wblk, bprime, WgT, wgb, Wt, rw = _prep_params(inputs)
    feat = np.asarray(inputs["features"], np.float32)
    bf = ml_dtypes.bfloat16

    rcol = np.full((128, 1), rw, np.float32)
    base = {
        "R1": consts["R1"], "R2a": consts["R2a"], "R2b": consts["R2b"],
        "RA1": consts["RA1"], "RA2": consts["RA2"], "RB1": consts["RB1"],
        "RB2": consts["RB2"], "G16": consts["G16"], "E16": consts["E16"],
        "F2": consts["F2"], "E4": consts["E4"], "maskJ": consts["maskJ"],
        "Wblk": Wblk, "bprime": bprime, "WgT": WgT, "wgb": wgb,
        "rcol": rcol,
    }
    in_maps = []
    for k in range(NCORES):
        sl = slice(k * CS, (k + 1) * CS)
        # featf: [h, (b, c, w)]
        ff = feat[:, sl].transpose(2, 0, 1, 3).reshape(128, B * CS * W).copy()
        # featg: [(t, c), (s)] with t = h-half
        fg = feat[k].reshape(C, 2, 64 * 128).transpose(1, 0, 2) \
                    .reshape(128, 64 * 128).copy()
        # ftiles: [J, (f, p), (c, ri, k2)]
        Wts = Wt[:, sl]                                   # [F, CS, H, WF]
        ftiles = np.empty((4, 128, CS * 2 * WF), np.float32)
        for J in range(4):
            blk = Wts[:, :, 32 * J:32 * J + 32, :]        # [F, CS, 32, WF]
            re = blk.real.astype(np.float32)
            im = blk.imag.astype(np.float32)
            # [(f,p), (c, ri, k2)]
            stacked = np.stack([re, im], axis=3)          # [F, CS, 32, 2, WF]
            ftiles[J] = stacked.transpose(0, 2, 1, 3, 4).reshape(128, CS * 2 * WF)
        m = dict(base)
        m["featf"] = ff
        m["featg"] = fg
        m["ftiles"] = ftiles
        in_maps.append(m)

    res = run_bass_kernel_spmd(nc, in_maps, list(range(NCORES)))
    out = np.empty((B, C, H, W), np.float32)
    for k in range(NCORES):
        out[:, k * CS:(k + 1) * CS] = res.results[k]["out"]
    return out


if __name__ == "__main__":
    import jax
    jax.config.update("jax_platforms", "cpu")



# revision 7
# speedup vs baseline: 1.6093x; 1.6093x over previous
"""Trainium2 Bass kernel for nn_FDSM_40295383171690 (restructured v2).

Math (validated in model.py / model_q.py, quantized rel err ~5e-3):
  gating: GN(concat(x,x)) == concat(GN4(x), GN4(x)); fold gamma/beta into the
          1x1 conv -> W', b'; fold GN scale into W rows at runtime;
          weights = softmax(wg @ GAP(relu(Wscaled x + b''))).
  fft (per chain = one (b,c) image):
    X1 = x @ Rw                  (rfft along w, k2 in [0,64])
    X  = Ch^T X1r + Sh^T X1i | Ch^T X1i - Sh^T X1r   (full DFT along h)
    Xbf = X/2 (bf16); Srp = Xbfr^2 - Xbfi^2 = (Xr^2-Xi^2)/4; P = XbfrXbfi
    Wm = sum_f w_f * (4*Wsym_f)  (masked matmul, K=(f,pp32), out rows 32J+pp)
    Dr = Srp*Wmr - P*Wmi + 2r*Xbfr ; Di = Srp*Wmi + P*Wmr + 2r*Xbfi
    z  = Dr^T RA1 + Di^T RA2     (iDFT along k1, out [k2, (zr|zi)])
    y  = zr^T Gc + zi^T Gs       (Hermitian-weighted iDFT along k2)
  The residual r*x is folded into D via the spectrum X (exact up to bf16).

Sharding: core k = gating for sample k (all C) + FFT branch for channels
[8k,8k+8) of all samples; the [8,4] gating weights are AllGathered on-chip.
"""

import numpy as np
import ml_dtypes

import concourse.bass as bass
import concourse.bacc as bacc
import concourse.mybir as mybir
import concourse.tile as tile
from concourse.bass_utils import run_bass_kernel_spmd

dt = mybir.dt
AF = mybir.ActivationFunctionType
ALU = mybir.AluOpType
AX = mybir.AxisListType

B, C, H, W, F = 8, 64, 128, 128, 4
WF = 65
NCORES = 8
CS = C // NCORES
EPS = 1e-5
HW = H * W

_cache = {}

# column offsets inside packed const tensors (all fp16 in c16)
C16_RW = 0          # [128,130]
C16_CH = 130        # [128,128]
C16_SH = 258
C16_SHN = 386
C16_MQ = 514        # [128,32] maskQ
C16_RA1 = 546       # [128,256] 64*RA1
C16_RA2 = 802       # [128,256] 64*RA2
C16_GC = 1058       # [65,128]
C16_GS = 1186       # [65,128]
C16_N = 1314
C32_WBLK = 0        # [128,128]
C32_G16 = 128       # [128,16]
C32_E16 = 144       # [16,128]
C32_F2 = 272        # [128,64]
C32_E4 = 336        # [4,128]
C32_BPR = 464       # [128,1]
C32_WGT = 465       # [64,4]
C32_WGB = 469       # [1,4]
C32_RC2 = 473       # [128,1]  residual_weight/32
C32_N = 474


def _build_constants():
    h = np.arange(H)
    k1 = np.arange(H)
    w = np.arange(W)
    k2 = np.arange(WF)
    th = 2 * np.pi * np.outer(w, k2) / W
    Rw = np.concatenate([np.cos(th), -np.sin(th)], 1)            # [128,130]
    ph = 2 * np.pi * np.outer(h, k1) / H
    Ch = np.cos(ph)
    Sh = np.sin(ph)
    phi = 2 * np.pi * np.outer(k1, h) / H
    Cih = np.cos(phi) / H
    Sih = np.sin(phi) / H
    RA1 = np.concatenate([Cih, Sih], 1)                          # [128,256]
    RA2 = np.concatenate([-Sih, Cih], 1)
    psi = 2 * np.pi * np.outer(k2, w) / W
    cj = np.ones(WF)
    cj[1:64] = 2.0
    Gc = (cj[:, None] * np.cos(psi)) / W                         # [65,128]
    Gs = (-cj[:, None] * np.sin(psi)) / W

    maskQ = np.zeros((128, 32), np.float32)
    for p in range(128):
        maskQ[p, p % 32] = 1.0

    c16 = np.zeros((128, C16_N), np.float16)
    c16[:, C16_RW:C16_RW + 130] = Rw.astype(np.float16)
    c16[:, C16_CH:C16_CH + 128] = Ch.astype(np.float16)
    c16[:, C16_SH:C16_SH + 128] = Sh.astype(np.float16)
    c16[:, C16_SHN:C16_SHN + 128] = (-Sh).astype(np.float16)
    c16[:, C16_MQ:C16_MQ + 32] = maskQ.astype(np.float16)
    c16[:, C16_RA1:C16_RA1 + 256] = (64.0 * RA1).astype(np.float16)
    c16[:, C16_RA2:C16_RA2 + 256] = (64.0 * RA2).astype(np.float16)
    c16[:WF, C16_GC:C16_GC + 128] = Gc.astype(np.float16)
    c16[:WF, C16_GS:C16_GS + 128] = Gs.astype(np.float16)

    G16 = np.zeros((128, 16), np.float32)
    E16 = np.zeros((16, 128), np.float32)
    for p in range(128):
        g = (p % 64) // 4
        G16[p, g] = 1.0
        E16[g, p] = 1.0
    F2 = np.zeros((128, 64), np.float32)
    for p in range(128):
        F2[p, p % 64] = 1.0 / HW
    E4 = np.zeros((4, 128), np.float32)
    for p in range(128):
        E4[p // 32, p] = 1.0

    c32 = np.zeros((128, C32_N), np.float32)
    c32[:, C32_G16:C32_G16 + 16] = G16
    c32[:16, C32_E16:C32_E16 + 128] = E16
    c32[:, C32_F2:C32_F2 + 64] = F2
    c32[:4, C32_E4:C32_E4 + 128] = E4
    return {"c16": c16, "c32": c32}


def _prep_params(inputs):
    gamma = np.asarray(inputs["gn_gamma"], np.float64)
    beta = np.asarray(inputs["gn_beta"], np.float64)
    agg_w = np.asarray(inputs["agg_w"], np.float64)
    agg_b = np.asarray(inputs["agg_b"], np.float64)
    wg_w = np.asarray(inputs["wg_w"], np.float64)
    wg_b = np.asarray(inputs["wg_b"], np.float64)

    Wp = agg_w[:, :C] * gamma[None, :C] + agg_w[:, C:] * gamma[None, C:]
    bp = agg_w[:, :C] @ beta[:C] + agg_w[:, C:] @ beta[C:] + agg_b
    Wblk = np.zeros((128, 128), np.float32)
    for t in range(2):
        Wblk[64 * t:64 * t + 64, 64 * t:64 * t + 64] = Wp.T.astype(np.float32)
    bprime = np.zeros((128, 1), np.float32)
    bprime[:64, 0] = bp.astype(np.float32)
    bprime[64:, 0] = bp.astype(np.float32)
    WgT = np.zeros((64, 4), np.float32)
    WgT[:, :] = wg_w.T.astype(np.float32)
    wgb = wg_b.astype(np.float32).reshape(1, 4)

    ds = np.asarray(inputs["ds_w"], np.float64)
    Wc = ds[..., 0] + 1j * ds[..., 1]                  # [F,C,H(k1),WF(k2)]
    rev = (-np.arange(H)) % H
    Wt = Wc.copy()
    for j in (0, WF - 1):
        Wt[..., j] = 0.5 * (Wc[..., j] + np.conj(Wc[:, :, rev, j]))
    Wt *= 4.0 / 64.0                                   # X/2 fold + D/64 range fold
    rw = float(np.asarray(inputs["residual_weight"]).ravel()[0])
    return Wblk, bprime, WgT, wgb, Wt, rw


def _build_kernel():
    bf16, f16, f32, f32r = dt.bfloat16, dt.float16, dt.float32, dt.float32r

    nc = bacc.Bacc("TRN2", target_bir_lowering=False, debug=False,
                   num_devices=NCORES)

    d = {}
    d["featg"] = nc.dram_tensor("featg", [128, 64 * 128], f16,
                                kind="ExternalInput").ap()
    d["featfw"] = nc.dram_tensor("featfw", [128, B * CS * H], f16,
                                 kind="ExternalInput").ap()
    d["ftiles"] = nc.dram_tensor("ftiles", [4, 128, CS * 2 * WF], f16,
                                 kind="ExternalInput").ap()
    d["c16"] = nc.dram_tensor("c16", [128, C16_N], f16,
                              kind="ExternalInput").ap()
    d["c32"] = nc.dram_tensor("c32", [128, C32_N], f32r,
                              kind="ExternalInput").ap()
    out_d = nc.dram_tensor("out", [B, CS, H, W], f32, kind="ExternalOutput").ap()

    with tile.TileContext(nc) as tc:
        with (
            tc.tile_pool(name="consts", bufs=1) as cp,
            tc.tile_pool(name="feat", bufs=1) as fp,
            tc.tile_pool(name="gate", bufs=1) as gp,
            tc.tile_pool(name="scr", bufs=2) as scp,
            tc.tile_pool(name="sgrp", bufs=1) as sg,
            tc.tile_pool(name="work", bufs=3) as wk,
            tc.tile_pool(name="outp", bufs=3) as op_,
            tc.tile_pool(name="ps_a", bufs=1, space="PSUM") as ps_a,
            tc.tile_pool(name="ps_b", bufs=2, space="PSUM") as ps_b,
            tc.tile_pool(name="ps_i", bufs=1, space="PSUM") as ps_i,
            tc.tile_pool(name="dram", bufs=1, space="DRAM") as dr,
        ):
            # ---------------- DMAs (SP queue, order = priority) -------------
            featg = fp.tile([128, 64 * 128], f16, tag="featg")
            for ch in range(4):
                nc.sync.dma_start(featg[:, ch * 2048:(ch + 1) * 2048],
                                  d["featg"][:, ch * 2048:(ch + 1) * 2048])
            c16t = cp.tile([128, C16_N], f16, tag="c16")
            nc.sync.dma_start(c16t[:], d["c16"][:])
            c32t = cp.tile([128, C32_N], f32r, tag="c32")
            nc.sync.dma_start(c32t[:], d["c32"][:])
            featfw = fp.tile([128, B * CS * H], f16, tag="featfw")
            for b in range(4):
                nc.sync.dma_start(
                    featfw[:, b * CS * H:(b + 1) * CS * H],
                    d["featfw"][:, b * CS * H:(b + 1) * CS * H])
            ftl = []
            for J in range(4):
                t = fp.tile([128, CS * 2 * WF], f16, tag=f"ftl{J}")
                nc.sync.dma_start(t[:], d["ftiles"][J])
                ftl.append(t)
            for b in range(4, 8):
                nc.sync.dma_start(
                    featfw[:, b * CS * H:(b + 1) * CS * H],
                    d["featfw"][:, b * CS * H:(b + 1) * CS * H])

            # const views
            Rw_v = c16t[:, C16_RW:C16_RW + 130]
            Ch_v = c16t[:, C16_CH:C16_CH + 128]
            Sh_v = c16t[:, C16_SH:C16_SH + 128]
            Shn_v = c16t[:, C16_SHN:C16_SHN + 128]
            mq_v = c16t[:, C16_MQ:C16_MQ + 32]
            Gc_v = c16t[0:WF, C16_GC:C16_GC + 128]
            Gs_v = c16t[0:WF, C16_GS:C16_GS + 128]
            RA1_v = c16t[:, C16_RA1:C16_RA1 + 256]
            RA2_v = c16t[:, C16_RA2:C16_RA2 + 256]
            Wblk_v = c32t[:, C32_WBLK:C32_WBLK + 128]
            G16_v = c32t[:, C32_G16:C32_G16 + 16]
            E16_v = c32t[0:16, C32_E16:C32_E16 + 128]
            F2_v = c32t[:, C32_F2:C32_F2 + 64]
            E4_v = c32t[0:4, C32_E4:C32_E4 + 128]
            bpr_v = c32t[:, C32_BPR:C32_BPR + 1]
            WgT_v = c32t[0:64, C32_WGT:C32_WGT + 4]
            wgb_v = c32t[0:1, C32_WGB:C32_WGB + 4]
            rc2_v = c32t[:, C32_RC2:C32_RC2 + 1]

            # ---------------- gating -----------------------------------------
            stats = gp.tile([128, 8], f32, tag="stats")
            for ch in range(4):
                fgch = featg[:, ch * 2048:(ch + 1) * 2048]
                sc = scp.tile([128, 2048], f16, tag="sc")
                nc.vector.tensor_scalar(sc[:], fgch, 1.0, 0.0, ALU.mult,
                                        ALU.add, accum_out=stats[:, ch:ch + 1])
                sc2 = scp.tile([128, 2048], f16, tag="sc2")
                if ch < 2:
                    nc.scalar.activation(sc2[:], fgch, AF.Square,
                                         accum_out=stats[:, 4 + ch:5 + ch])
                else:
                    nc.vector.tensor_tensor_reduce(
                        sc2[:], fgch, fgch, 1.0, 0.0, ALU.mult, ALU.add,
                        accum_out=stats[:, 4 + ch:5 + ch])
            stats2 = gp.tile([128, 2], f32r, tag="stats2")
            with nc.allow_low_precision(reason="f32r is fp32-width"):
                nc.vector.tensor_reduce(stats2[:, 0:1], stats[:, 0:4], AX.X,
                                        ALU.add)
                nc.vector.tensor_reduce(stats2[:, 1:2], stats[:, 4:8], AX.X,
                                        ALU.add)
            gstat = ps_i.tile([16, 2], f32, tag="pB")
            nc.tensor.matmul(gstat[:], G16_v, stats2[:])
            gs = gp.tile([16, 6], f32, tag="gs")
            nc.scalar.mul(gs[:, 0:1], gstat[:, 0:1], 1.0 / (4 * HW))
            nc.scalar.mul(gs[:, 1:2], gstat[:, 1:2], 1.0 / (4 * HW))
            nc.scalar.activation(gs[:, 2:3], gs[:, 0:1], AF.Square)
            nc.vector.tensor_sub(gs[:, 3:4], gs[:, 1:2], gs[:, 2:3])
            epst = gp.tile([16, 1], f32, tag="epst")
            nc.vector.memset(epst[:], EPS)
            nc.scalar.activation(gs[:, 4:5], gs[:, 3:4], AF.Sqrt,
                                 bias=epst[:, 0:1])
            nc.vector.reciprocal(gs[:, 5:6], gs[:, 4:5])
            gs2 = gp.tile([16, 2], f32r, tag="gs2")
            nc.vector.tensor_mul(gs2[:, 0:1], gs[:, 0:1], gs[:, 5:6])
            nc.vector.tensor_scalar_mul(gs2[:, 0:1], gs2[:, 0:1], -1.0)
            nc.vector.tensor_copy(gs2[:, 1:2], gs[:, 5:6])
            pstat = ps_i.tile([128, 2], f32, tag="pB")
            nc.tensor.matmul(pstat[:], E16_v, gs2[:])
            nstat = gp.tile([128, 2], f32, tag="nstat")
            nc.scalar.copy(nstat[:], pstat[:])
            # Wscaled[c,o] = Wblk[c,o] * rsig_c ; b'' = Wblk^T bias + b'
            wsc = gp.tile([128, 128], f16, tag="wsc")
            nc.vector.tensor_scalar(wsc[:], Wblk_v, nstat[:, 1:2], None,
                                    ALU.mult)
            pbias = ps_i.tile([128, 1], f32, tag="pB")
            nc.tensor.matmul(pbias[:], Wblk_v,
                             nstat[:, 0:1].bitcast(f32r))
            btot = gp.tile([128, 1], f32, tag="btot")
            nc.vector.tensor_add(btot[:], pbias[:], bpr_v)
            gap = gp.tile([128, 16], f32, tag="gap")
            for j in range(16):
                yp = ps_b.tile([128, 512], f32, tag="yp")
                nc.tensor.matmul(yp[:], wsc[:], featg[:, j * 512:(j + 1) * 512])
                scr = scp.tile([128, 512], f16, tag="screlu")
                if j % 2 == 0:
                    nc.scalar.activation(scr[:], yp[:], AF.Relu,
                                         bias=btot[:, 0:1], scale=1.0,
                                         accum_out=gap[:, j:j + 1])
                else:
                    nc.vector.tensor_scalar(scr[:], yp[:], btot[:, 0:1], 0.0,
                                            ALU.add, ALU.max,
                                            accum_out=gap[:, j:j + 1])
            gsum = gp.tile([128, 1], f32r, tag="gsum")
            with nc.allow_low_precision(reason="f32r is fp32-width"):
                nc.vector.tensor_reduce(gsum[:], gap[:], AX.X, ALU.add)
            ppool = ps_i.tile([64, 1], f32, tag="pB")
            nc.tensor.matmul(ppool[:], F2_v, gsum[:])
            pooled = gp.tile([64, 1], f32r, tag="pooled")
            nc.scalar.copy(pooled[:], ppool[:])
            plog = ps_i.tile([1, 4], f32, tag="pB")
            nc.tensor.matmul(plog[:], pooled[:], WgT_v)
            logit = gp.tile([1, 8], f32, tag="logit")
            nc.vector.memset(logit[:], 0.0)
            nc.vector.tensor_add(logit[:, 0:4], plog[:], wgb_v)
            nc.vector.tensor_reduce(logit[:, 4:5], logit[:, 0:4], AX.X, ALU.max)
            nc.vector.tensor_scalar(logit[:, 0:4], logit[:, 0:4],
                                    logit[:, 4:5], None, ALU.subtract)
            wrow = gp.tile([1, 4], f32, tag="wrow")
            nc.scalar.activation(wrow[:], logit[:, 0:4], AF.Exp,
                                 accum_out=logit[:, 5:6])
            nc.vector.reciprocal(logit[:, 6:7], logit[:, 5:6])
            nc.vector.tensor_scalar(wrow[:], wrow[:], logit[:, 6:7], None,
                                    ALU.mult)
            ag_in = dr.tile([1, 4], f32)
            ag_out = dr.tile([8, 4], f32)
            nc.sync.dma_start(ag_in[:], wrow[:])
            nc.gpsimd.collective_compute(
                "AllGather", ALU.bypass, ins=[ag_in.opt()],
                outs=[ag_out.opt()],
                replica_groups=[list(range(NCORES))],
            )
            wT = gp.tile([4, 8], f32, tag="wT")
            nc.sync.dma_start(wT[:], ag_out[:].rearrange("b f -> f b"))
            pwcol = ps_i.tile([128, 8], f32, tag="pB")
            nc.tensor.matmul(pwcol[:], E4_v, wT[:].bitcast(f32r))
            wcol = gp.tile([128, 8], f32, tag="wcol")
            nc.scalar.copy(wcol[:], pwcol[:])
            wq = []
            for b in range(B):
                t = gp.tile([128, 32], f16, tag=f"wq{b}")
                nc.vector.tensor_scalar(t[:], mq_v, wcol[:, b:b + 1], None,
                                        ALU.mult)
                wq.append(t)

            # ---------------- FFT phase 1 (weight-independent) ---------------
            # group gi = (b, g): chains (b, 4g+q), q=0..3
            Xbf = []
            Srp = []
            Pp = []
            for gi in range(16):
                b, g = gi // 2, gi % 2
                xb = sg.tile([128, 520], bf16, tag=f"xbf{gi}")
                for half in range(2):
                    p1 = ps_a.tile([128, 260], f32, tag="st1")
                    for j in range(2):
                        c = 4 * g + 2 * half + j
                        col = (b * CS + c) * 128
                        nc.tensor.matmul(p1[:, j * 130:(j + 1) * 130],
                                         featfw[:, col:col + 128], Rw_v)
                    x1t = wk.tile([128, 260], f16, tag="x1t")
                    if half == 0:
                        nc.scalar.copy(x1t[:], p1[:])
                    else:
                        nc.vector.tensor_copy(x1t[:], p1[:])
                    x1v = x1t[:].rearrange("p (q x) -> p q x", q=2)
                    p2 = ps_b.tile([128, 260], f32, tag="st2")
                    p2v = p2[:].rearrange("p (q x) -> p q x", q=2)
                    nc.tensor.matmul(p2v[:, :, 0:65], Ch_v, x1v[:, :, 0:65],
                                     start=True, stop=False)
                    nc.tensor.matmul(p2v[:, :, 0:65], Sh_v, x1v[:, :, 65:130],
                                     start=False, stop=True)
                    nc.tensor.matmul(p2v[:, :, 65:130], Ch_v, x1v[:, :, 65:130],
                                     start=True, stop=False)
                    nc.tensor.matmul(p2v[:, :, 65:130], Shn_v, x1v[:, :, 0:65],
                                     start=False, stop=True)
                    xh = xb[:, half * 260:(half + 1) * 260]
                    if half == 0:
                        nc.scalar.mul(xh, p2[:], 0.5)
                    else:
                        nc.vector.tensor_scalar_mul(xh, p2[:], 0.5)
                xv = xb[:].rearrange("p (q x) -> p q x", q=4)
                xr = xv[:, :, 0:65]
                xi = xv[:, :, 65:130]
                sq1 = wk.tile([128, 260], bf16, tag="sq1")
                sq2 = wk.tile([128, 260], bf16, tag="sq2")
                sq1v = sq1[:].rearrange("p (q x) -> p q x", q=4)
                sq2v = sq2[:].rearrange("p (q x) -> p q x", q=4)
                nc.vector.tensor_mul(sq1v, xr, xr)
                nc.vector.tensor_mul(sq2v, xi, xi)
                pt = sg.tile([128, 260], bf16, tag=f"pp{gi}")
                ptv = pt[:].rearrange("p (q x) -> p q x", q=4)
                nc.vector.scalar_tensor_tensor(ptv, xr, 2.0, xi,
                                               ALU.mult, ALU.mult)
                st = sg.tile([128, 260], bf16, tag=f"srp{gi}")
                nc.vector.tensor_sub(st[:], sq1[:], sq2[:])
                Xbf.append(xb)
                Srp.append(st)
                Pp.append(pt)

            # ---------------- FFT phase 2 (weight-dependent) ------------------
            for gi in range(16):
                b, g = gi // 2, gi % 2
                wm = wk.tile([128, 520], f16, tag="wm")
                for half in range(2):
                    pm = ps_a.tile([128, 260], f32, tag="pm")
                    fcol = (4 * g + 2 * half) * 130
                    for J in range(4):
                        nc.tensor.matmul(pm[32 * J:32 * J + 32, :], wq[b],
                                         ftl[J][:, fcol:fcol + 260],
                                         tile_position=(0, 32 * J))
                    if half == 0:
                        nc.scalar.copy(wm[:, 0:260], pm[:])
                    else:
                        nc.scalar.copy(wm[:, 260:520], pm[:])
                wmv = wm[:].rearrange("p (q x) -> p q x", q=4)
                wmr = wmv[:, :, 0:65]
                wmi = wmv[:, :, 65:130]
                srv = Srp[gi][:].rearrange("p (q x) -> p q x", q=4)
                ppv = Pp[gi][:].rearrange("p (q x) -> p q x", q=4)
                xv = Xbf[gi][:].rearrange("p (q x) -> p q x", q=4)
                t1 = wk.tile([128, 260], bf16, tag="t1")
                t2 = wk.tile([128, 260], bf16, tag="t2")
                t1v = t1[:].rearrange("p (q x) -> p q x", q=4)
                t2v = t2[:].rearrange("p (q x) -> p q x", q=4)
                drt = wk.tile([128, 260], f16, tag="drt")
                dit = wk.tile([128, 260], f16, tag="dit")
                # Dr = Srp*Wmr - P*Wmi + 2r*Xbfr
                nc.vector.tensor_mul(t1v, srv, wmr)
                nc.vector.tensor_mul(t2v, ppv, wmi)
                nc.vector.tensor_sub(t1[:], t1[:], t2[:])
                nc.gpsimd.scalar_tensor_tensor(
                    drt[:].rearrange("p (q x) -> p q x", q=4),
                    xv[:, :, 0:65], rc2_v[:, 0:1].bitcast(f32), t1v,
                    ALU.mult, ALU.add)
                # Di = Srp*Wmi + P*Wmr + 2r*Xbfi
                nc.vector.tensor_mul(t2v, srv, wmi)
                nc.vector.tensor_mul(t1v, ppv, wmr)
                nc.vector.tensor_add(t2[:], t2[:], t1[:])
                nc.gpsimd.scalar_tensor_tensor(
                    dit[:].rearrange("p (q x) -> p q x", q=4),
                    xv[:, :, 65:130], rc2_v[:, 0:1].bitcast(f32), t2v,
                    ALU.mult, ALU.add)
                ot = op_.tile([128, 512], f32, tag="ot")
                for half in range(2):
                    pA = ps_i.tile([65, 512], f32, tag="pA")
                    for j in range(2):
                        q = 2 * half + j
                        nc.tensor.matmul(pA[:, j * 256:(j + 1) * 256],
                                         drt[:, q * 65:(q + 1) * 65], RA1_v,
                                         start=True, stop=False)
                        nc.tensor.matmul(pA[:, j * 256:(j + 1) * 256],
                                         dit[:, q * 65:(q + 1) * 65], RA2_v,
                                         start=False, stop=True)
                    z2 = wk.tile([65, 512], f16, tag="z2")
                    if half == 0:
                        nc.scalar.copy(z2[:], pA[:])
                    else:
                        nc.vector.tensor_copy(z2[:], pA[:])
                    pB = ps_i.tile([128, 256], f32, tag="pB")
                    for j in range(2):
                        nc.tensor.matmul(pB[:, j * 128:(j + 1) * 128],
                                         z2[:, j * 256:j * 256 + 128], Gc_v,
                                         start=True, stop=False)
                        nc.tensor.matmul(pB[:, j * 128:(j + 1) * 128],
                                         z2[:, j * 256 + 128:(j + 1) * 256],
                                         Gs_v, start=False, stop=True)
                    oh = ot[:, half * 256:(half + 1) * 256]
                    if half == 0:
                        nc.scalar.copy(oh, pB[:])
                    else:
                        nc.vector.tensor_copy(oh, pB[:])
                nc.sync.dma_start(
                    out_d[b, 4 * g:4 * g + 4].rearrange("c h w -> h c w"),
                    ot[:].rearrange("p (c w) -> p c w", c=4))
    nc.compile()
    return nc


def _get_kernel():
    if "nc" not in _cache:
        _cache["nc"] = _build_kernel()
        _cache["consts"] = _build_constants()
    return _cache["nc"], _cache["consts"]


def kernel(**inputs):
    nc, consts = _get_kernel()
    Wblk, bprime, WgT, wgb, Wt, rw = _prep_params(inputs)
    feat = np.asarray(inputs["features"], np.float32)
    bf = ml_dtypes.bfloat16

    c32 = consts["c32"].copy()
    c32[:, C32_WBLK:C32_WBLK + 128] = Wblk
    c32[:, C32_BPR:C32_BPR + 1] = bprime
    c32[:64, C32_WGT:C32_WGT + 4] = WgT
    c32[:1, C32_WGB:C32_WGB + 4] = wgb
    c32[:, C32_RC2:C32_RC2 + 1] = rw / 32.0

    base = {"c16": consts["c16"], "c32": c32}
    in_maps = []
    for k in range(NCORES):
        sl = slice(k * CS, (k + 1) * CS)
        fg = feat[k].reshape(C, 2, 64 * 128).transpose(1, 0, 2) \
                    .reshape(128, 64 * 128).astype(np.float16)
        ff = feat[:, sl].transpose(3, 0, 1, 2).reshape(128, B * CS * H) \
                        .astype(np.float16)
        Wts = Wt[:, sl]                                   # [F, CS, H, WF]
        ftiles = np.empty((4, 128, CS * 2 * WF), np.float16)
        for J in range(4):
            blk = Wts[:, :, 32 * J:32 * J + 32, :]        # [F, CS, 32, WF]
            stacked = np.stack([blk.real, blk.imag], axis=3)
            ftiles[J] = stacked.transpose(0, 2, 1, 3, 4) \
                               .reshape(128, CS * 2 * WF).astype(np.float16)
        m = dict(base)
        m["featg"] = fg
        m["featfw"] = ff
        m["ftiles"] = ftiles
        in_maps.append(m)

    res = run_bass_kernel_spmd(nc, in_maps, list(range(NCORES)))
    out = np.empty((B, C, H, W), np.float32)
    for k in range(NCORES):
        out[:, k * CS:(k + 1) * CS] = res.results[k]["out"]
    return out
